# revision 5
# baseline (speedup 1.0000x reference)
"""Cross-attention + FFN + layernorm block on 8 Trainium2 NeuronCores.

Sharding: data-parallel over (B=4) x (LQ split in 2) -> 8 shards of 1024
query rows. Keys/values/weights are replicated per batch; each core runs
the full pipeline for its shard, so no collectives are needed.

Per-core pipeline (all layouts chosen so reductions stay on the free dim
and the softmax mask is a per-partition activation bias):
  1. PE-transpose input tiles -> feature-major x^T chunks.
  2. q^T/k^T (bf16, feature-major) and v (row-major, bf16) projections in
     fp32r with fp32 PSUM accumulation.
  3. scores^T[k,q] = k^T-chunk.T @ q^T (per head); exp via ACT with the
     -1e6 mask bias per k partition; o[q,65] = exp^T.T @ [v | ones]
     accumulated over k chunks -> column 64 is the softmax denominator.
  4. o -> o^T -> att = o @ Wo (row-major), att^T -> h^T = relu(att@W1+b1),
     ffn = h@W2, y = ffn + b2 + att, layernorm over the free dim.
"""

import sys

if '/opt/trn_rl_repo' not in sys.path:
    sys.path.insert(0, '/opt/trn_rl_repo')

import numpy as np

B, LQ, LK, D, H = 4, 2048, 2048, 768, 12
DH = D // H            # 64
NC = 8                 # cores
LQC = B * LQ // NC     # 1024 query rows per core
QB = LQC // 128        # 8 q row-tiles
KT = LK // 128         # 16 k row-tiles
C = D // 128           # 6 feature chunks
EPS = 1e-5

_CACHE = {}


def _build():
    import concourse.bacc as bacc
    import concourse.bass as bass
    import concourse.tile as tile
    import concourse.mybir as mybir
    from concourse.masks import make_identity

    f32 = mybir.dt.float32
    f32r = mybir.dt.float32r
    bf16 = mybir.dt.bfloat16
    Exp = mybir.ActivationFunctionType.Exp
    Relu = mybir.ActivationFunctionType.Relu
    Sqrt = mybir.ActivationFunctionType.Sqrt

    nc = bacc.Bacc("TRN2", target_bir_lowering=False, debug=False)

    xq = nc.dram_tensor("xq", [LQC, D], f32, kind="ExternalInput")
    xk = nc.dram_tensor("xk", [LK, D], f32, kind="ExternalInput")
    xv = nc.dram_tensor("xv", [LK, D], f32, kind="ExternalInput")
    mbias = nc.dram_tensor("mbias", [128, KT], f32, kind="ExternalInput")
    wq = nc.dram_tensor("wq", [D, D], f32, kind="ExternalInput")
    wk = nc.dram_tensor("wk", [D, D], f32, kind="ExternalInput")
    wv = nc.dram_tensor("wv", [D, D], f32, kind="ExternalInput")
    wo = nc.dram_tensor("wo", [D, D], f32, kind="ExternalInput")
    w1 = nc.dram_tensor("w1", [D, D], f32, kind="ExternalInput")
    w2 = nc.dram_tensor("w2", [D, D], f32, kind="ExternalInput")
    b1c = nc.dram_tensor("b1c", [128, C], f32, kind="ExternalInput")
    b2v = nc.dram_tensor("b2v", [D], f32, kind="ExternalInput")
    gv = nc.dram_tensor("gv", [D], f32, kind="ExternalInput")
    bv = nc.dram_tensor("bv", [D], f32, kind="ExternalInput")
    yout = nc.dram_tensor("yout", [LQC, D], f32, kind="ExternalOutput")

    def wcol_ap(w, n):
        # lhsT chunk column [128(din part), C, 128(dout)] of a [D, D] weight
        return w.ap().rearrange("(c p) n -> p c n", p=128)[:, :, n * 128:(n + 1) * 128]

    def wrow_ap(w):
        # rhs layout [128(din part), C, D]
        return w.ap().rearrange("(c p) n -> p c n", p=128)

    def bcast_ap(v):
        a = v.ap()
        return bass.AP(tensor=a.tensor, offset=a.offset, ap=[[0, 128]] + list(a.ap))

    with tile.TileContext(nc) as tc:
        with tc.tile_pool(name="consts", bufs=1) as consts, \
             tc.tile_pool(name="persist", bufs=1) as persist, \
             tc.tile_pool(name="work", bufs=3) as work, \
             tc.tile_pool(name="wchunk", bufs=2) as wchunk, \
             tc.tile_pool(name="pp_ab", bufs=2, space="PSUM") as pp_ab, \
             tc.tile_pool(name="pp_512", bufs=2, space="PSUM") as pp_512, \
             tc.tile_pool(name="pp_acc", bufs=4, space="PSUM") as pp_acc:
            pp_t = pp_ab
            pp_256 = pp_512

            ident = consts.tile([128, 128], f32)
            make_identity(nc, ident)
            mb = consts.tile([128, KT], f32)
            nc.sync.dma_start(out=mb, in_=mbias.ap())
            b1_t = consts.tile([128, C], f32)
            nc.sync.dma_start(out=b1_t, in_=b1c.ap())
            b2_t = consts.tile([128, D], f32)
            nc.gpsimd.dma_start(out=b2_t, in_=bcast_ap(b2v))
            g_t = consts.tile([128, D], f32)
            nc.gpsimd.dma_start(out=g_t, in_=bcast_ap(gv))
            be_t = consts.tile([128, D], f32)
            nc.gpsimd.dma_start(out=be_t, in_=bcast_ap(bv))
            eps_t = consts.tile([128, 1], f32)
            nc.vector.memset(eps_t, EPS)

            # persistent activations; tags pair tensors with disjoint
            # lifetimes so they share one SBUF slot (attention phase dies
            # before the FFN phase starts)
            qT = persist.tile([128, C, LQC], bf16, tag="slotC")
            kT = persist.tile([128, C, LK], bf16, tag="slotA")
            vp = persist.tile([128, KT, H, DH + 1], bf16, tag="slotB")
            o_sb = persist.tile([128, QB, D], f32r, tag="slotD")
            wv_t = persist.tile([128, C, D], f32r, tag="wmat")

            def transpose_cols(src_ap, dst_tile, dst_q0, qw):
                """PE-transpose [qw(part), D] row-major -> dst[:, c, dst_q0:+qw]."""
                for c in range(C):
                    pt = pp_t.tile([128, 128], f32, tag="pab")
                    nc.tensor.transpose(
                        pt[:, 0:qw], src_ap[:, c * 128:(c + 1) * 128], ident[:])
                    nc.vector.tensor_copy(
                        out=dst_tile[:, c, dst_q0:dst_q0 + qw],
                        in_=pt[:, 0:qw])

            # ---- q/k projections: process two 128-row tiles (256 cols) at a time
            def proj_T(x_dram, nrows, w_dram, out_tile):
                nt = nrows // 256
                for t in range(nt):
                    xt = work.tile([128, 2, D], f32, tag="xt")
                    nc.sync.dma_start(
                        out=xt, in_=x_dram.ap().rearrange(
                            "(t two p) d -> t two p d", two=2, p=128)[t].rearrange(
                            "two p d -> p two d"))
                    xT = work.tile([128, C, 256], f32r, tag="xT")
                    for two in range(2):
                        for c in range(C):
                            pt = pp_t.tile([128, 128], f32, tag="pab")
                            nc.tensor.transpose(
                                pt[:], xt[:, two, c * 128:(c + 1) * 128], ident[:])
                            nc.vector.tensor_copy(
                                out=xT[:, c, two * 128:(two + 1) * 128], in_=pt[:])
                    for n in range(C):
                        wcol = wchunk.tile([128, C, 128], f32r, tag="wcol")
                        nc.sync.dma_start(out=wcol, in_=wcol_ap(w_dram, n).bitcast(f32r))
                        ps = pp_512.tile([128, 256], f32, tag="p512")
                        for c in range(C):
                            nc.tensor.matmul(ps[:], wcol[:, c, :], xT[:, c, :],
                                             start=(c == 0), stop=(c == C - 1))
                        nc.vector.tensor_copy(
                            out=out_tile[:, n, t * 256:(t + 1) * 256], in_=ps[:])

            proj_T(xq, LQC, wq, qT)
            proj_T(xk, LK, wk, kT)

            # ---- v projection: row-major out [128(kpos), h, 64] + ones column
            nc.sync.dma_start(out=wv_t, in_=wrow_ap(wv).bitcast(f32r))
            for t in range(KT):
                xt = work.tile([128, D], f32, tag="xt")
                nc.sync.dma_start(
                    out=xt, in_=xv.ap()[t * 128:(t + 1) * 128, :])
                xT = work.tile([128, C, 128], f32r, tag="xT")
                for c in range(C):
                    pt = pp_t.tile([128, 128], f32, tag="pab")
                    nc.tensor.transpose(pt[:], xt[:, c * 128:(c + 1) * 128], ident[:])
                    nc.vector.tensor_copy(out=xT[:, c, :], in_=pt[:])
                for n0, nw in ((0, 512), (512, 256)):
                    pool = pp_512 if nw == 512 else pp_256
                    ps = pool.tile([128, nw], f32, tag="p512")
                    for c in range(C):
                        nc.tensor.matmul(ps[:], xT[:, c, :],
                                         wv_t[:, c, n0:n0 + nw],
                                         start=(c == 0), stop=(c == C - 1))
                    h0 = n0 // DH
                    nc.vector.tensor_copy(
                        out=vp[:, t, h0:h0 + nw // DH, 0:DH],
                        in_=ps[:].rearrange("p (h d) -> p h d", d=DH))
            nc.vector.memset(vp[:, :, :, DH:DH + 1], 1.0)

            # ---- attention core
            for qc in range(2):
                for h in range(H):
                    p0 = (h % 2) * 64
                    cc = h // 2
                    pos = [pp_acc.tile([128, DH + 1], f32, tag="po",
                                       name=f"po_{qc}_{h}_{i}") for i in range(4)]
                    for kc in range(KT):
                        ps_s = pp_512.tile([128, 512], f32, tag="p512")
                        nc.tensor.matmul(
                            ps_s[:],
                            kT[p0:p0 + 64, cc, kc * 128:(kc + 1) * 128],
                            qT[p0:p0 + 64, cc, qc * 512:(qc + 1) * 512],
                            start=True, stop=True)
                        ex = work.tile([128, 512], bf16, tag="ex")
                        nc.scalar.activation(out=ex[:], in_=ps_s[:], func=Exp,
                                             bias=mb[:, kc:kc + 1], scale=1.0)
                        for qs in range(4):
                            nc.tensor.matmul(
                                pos[qs][:],
                                ex[:, qs * 128:(qs + 1) * 128],
                                vp[:, kc, h, :],
                                start=(kc == 0), stop=(kc == KT - 1))
                    for qs in range(4):
                        rec = work.tile([128, 1], f32, tag="rec")
                        nc.vector.reciprocal(rec[:], pos[qs][:, DH:DH + 1])
                        nc.vector.tensor_scalar_mul(
                            out=o_sb[:, qc * 4 + qs, h * DH:(h + 1) * DH],
                            in0=pos[qs][:, 0:DH],
                            scalar1=rec[:])

            # ---- o^T (reuses vp's slot; attention is complete here)
            oT = persist.tile([128, C, LQC], f32r, tag="slotB")
            for qb in range(QB):
                transpose_cols(o_sb[:, qb, :].bitcast(f32), oT, qb * 128, 128)

            # ---- att = o @ Wo (row-major out)
            wo_t = persist.tile([128, C, D], f32r, tag="wmat")
            nc.sync.dma_start(out=wo_t, in_=wrow_ap(wo).bitcast(f32r))
            att = persist.tile([128, QB, D], f32, tag="slotA")
            for qb in range(QB):
                for n0, nw in ((0, 512), (512, 256)):
                    pool = pp_512 if nw == 512 else pp_256
                    ps = pool.tile([128, nw], f32, tag="p512")
                    for c in range(C):
                        nc.tensor.matmul(
                            ps[:], oT[:, c, qb * 128:(qb + 1) * 128],
                            wo_t[:, c, n0:n0 + nw],
                            start=(c == 0), stop=(c == C - 1))
                    nc.vector.tensor_copy(out=att[:, qb, n0:n0 + nw], in_=ps[:])

            # ---- att^T (reuses qT's slot)
            attT = persist.tile([128, C, LQC], f32r, tag="slotC")
            for qb in range(QB):
                transpose_cols(att[:, qb, :], attT, qb * 128, 128)

            # ---- h^T = relu(att @ W1 + b1)^T (reuses o_sb's slot)
            hT = persist.tile([128, C, LQC], f32r, tag="slotD")
            for n in range(C):
                w1col = wchunk.tile([128, C, 128], f32r, tag="wcol")
                nc.sync.dma_start(out=w1col, in_=wcol_ap(w1, n).bitcast(f32r))
                for qc in range(2):
                    ps = pp_512.tile([128, 512], f32, tag="p512")
                    for c in range(C):
                        nc.tensor.matmul(
                            ps[:], w1col[:, c, :], attT[:, c, qc * 512:(qc + 1) * 512],
                            start=(c == 0), stop=(c == C - 1))
                    nc.scalar.activation(
                        out=hT[:, n, qc * 512:(qc + 1) * 512], in_=ps[:],
                        func=Relu, bias=b1_t[:, n:n + 1], scale=1.0)

            # ---- ffn + residual + layernorm
            w2_t = persist.tile([128, C, D], f32r, tag="wmat")
            nc.sync.dma_start(out=w2_t, in_=wrow_ap(w2).bitcast(f32r))
            for qb in range(QB):
                y = work.tile([128, D], f32, tag="y")
                for n0, nw in ((0, 512), (512, 256)):
                    pool = pp_512 if nw == 512 else pp_256
                    ps = pool.tile([128, nw], f32, tag="p512")
                    for c in range(C):
                        nc.tensor.matmul(
                            ps[:], hT[:, c, qb * 128:(qb + 1) * 128],
                            w2_t[:, c, n0:n0 + nw],
                            start=(c == 0), stop=(c == C - 1))
                    nc.vector.tensor_add(out=y[:, n0:n0 + nw], in0=ps[:],
                                         in1=att[:, qb, n0:n0 + nw])
                nc.vector.tensor_add(out=y[:], in0=y[:], in1=b2_t[:])
                stats = work.tile([128, 3, 6], f32, tag="stats")
                for sg in range(3):
                    nc.vector.bn_stats(out=stats[:, sg, :],
                                       in_=y[:, sg * 256:(sg + 1) * 256])
                mv = work.tile([128, 2], f32, tag="mv")
                nc.vector.bn_aggr(out=mv[:], in_=stats[:])
                rstd = work.tile([128, 1], f32, tag="rstd")
                nc.scalar.activation(out=rstd[:], in_=mv[:, 1:2], func=Sqrt,
                                     bias=eps_t[:], scale=1.0)
                nc.vector.reciprocal(rstd[:], rstd[:])
                yn = work.tile([128, D], f32, tag="yn")
                nc.vector.tensor_scalar(
                    out=yn[:], in0=y[:], scalar1=mv[:, 0:1], scalar2=rstd[:],
                    op0=mybir.AluOpType.subtract, op1=mybir.AluOpType.mult)
                nc.vector.tensor_mul(out=yn[:], in0=yn[:], in1=g_t[:])
                nc.vector.tensor_add(out=yn[:], in0=yn[:], in1=be_t[:])
                nc.sync.dma_start(out=yout.ap()[qb * 128:(qb + 1) * 128, :],
                                  in_=yn[:])

    nc.compile()
    return nc


def _get_nc():
    if "nc" not in _CACHE:
        _CACHE["nc"] = _build()
    return _CACHE["nc"]


def _prepare_in_maps(queries, keys, values, mask, Wq, Wk, Wv, Wo, W1, b1,
                     W2, b2, ln_g, ln_b):
    queries = np.asarray(queries, dtype=np.float32)
    keys = np.asarray(keys, dtype=np.float32)
    values = np.asarray(values, dtype=np.float32)
    mask = np.asarray(mask)

    valid = (mask != 0).sum(axis=1).astype(np.int64)        # [B]
    kidx = np.arange(LK)
    # additive exp-bias, [128, KT] per batch: index = kc*128 + p
    mb_all = np.where(kidx[None, :] < valid[:, None], 0.0, -1e6).astype(np.float32)
    mb_all = mb_all.reshape(B, KT, 128).transpose(0, 2, 1).copy()

    wq_s = (np.asarray(Wq, np.float32) / np.sqrt(np.float32(DH))).astype(np.float32)
    common = {
        "wq": wq_s,
        "wk": np.ascontiguousarray(Wk, np.float32),
        "wv": np.ascontiguousarray(Wv, np.float32),
        "wo": np.ascontiguousarray(Wo, np.float32),
        "w1": np.ascontiguousarray(W1, np.float32),
        "w2": np.ascontiguousarray(W2, np.float32),
        "b1c": np.ascontiguousarray(
            np.asarray(b1, np.float32).reshape(C, 128).T),
        "b2v": np.ascontiguousarray(b2, np.float32),
        "gv": np.ascontiguousarray(ln_g, np.float32),
        "bv": np.ascontiguousarray(ln_b, np.float32),
    }

    in_maps = []
    for core in range(NC):
        b, half = core // 2, core % 2
        in_maps.append(dict(
            common,
            xq=np.ascontiguousarray(queries[b, half * LQC:(half + 1) * LQC, :]),
            xk=np.ascontiguousarray(keys[b]),
            xv=np.ascontiguousarray(values[b]),
            mbias=np.ascontiguousarray(mb_all[b]),
        ))
    return in_maps


def kernel(queries, keys, values, mask, Wq, Wk, Wv, Wo, W1, b1, W2, b2,
           ln_g, ln_b, _trace=False):
    from concourse.bass_utils import run_bass_kernel_spmd

    in_maps = _prepare_in_maps(queries, keys, values, mask, Wq, Wk, Wv, Wo,
                               W1, b1, W2, b2, ln_g, ln_b)
    nc = _get_nc()
    res = run_bass_kernel_spmd(nc, in_maps, core_ids=list(range(NC)),
                               trace=_trace)
    _CACHE["last_result"] = res

    out = np.empty((B, LQ, D), dtype=np.float32)
    for core in range(NC):
        b, half = core // 2, core % 2
        out[b, half * LQC:(half + 1) * LQC, :] = res.results[core]["yout"]
    return out
